# revision 30
# baseline (speedup 1.0000x reference)
"""Trainium2 Bass kernel for nn_MemoryCore (sparse_attention).

Reference computation per batch b (B=8, T=4, De=128, Do=512, H=W=32):
    mi   = m_in transposed to (THW=4096, De=128)        # keys
    qi   = q_in as (De=128, HW=1024)                    # queries
    s    = mi @ qi / sqrt(De)                           # (4096, 1024)
    p    = softmax(s, axis=0)                           # over THW
    mo   = m_out raw-reshaped to (Do=512, THW=4096)
    mem  = mo @ p                                       # (512, 1024)
    out  = concat([mem.reshape(512,32,32), q_out])      # (1024, 32, 32)
    returns (out, p)

Sharding: pure data-parallel, one batch per NeuronCore (8 cores).

Kernel strategy (per core):
  - bmm1 computed in (k, n) layout: s[k,n] tiles of (128, 512), lhsT =
    mi_T[:, kblock] (d on partitions), rhs = qi[:, nhalf].  fp32r matmuls
    (1 cycle/row at N=512, ~1e-3 precision - far better than bf16).
  - softmax over k without max subtraction (scores ~ N(0,1); exp is safe
    in fp32).  e = exp(s/sqrt(De)) via ScalarE straight out of PSUM.
  - denominators: ones(128,1)^T @ e accumulated over all 32 k-tiles on PE.
  - bmm2: mem[d,n] accumulated in PSUM over k-tiles; lhsT = moT blocks
    produced by PE transposes of the natural-layout mo.
  - column scaling by 1/sums applied at the end to both p (the big output)
    and mem, using a PE-broadcast (1,512)->(128,512) reciprocal row.
"""

import math
import os
import sys

for _p in ("/opt/trn_rl_repo",):
    if os.path.isdir(_p) and _p not in sys.path:
        sys.path.insert(0, _p)

import numpy as np

import concourse.bass as bass
import concourse.mybir as mybir
import concourse.tile as tile
from concourse import bacc, bass_utils
from concourse.masks import make_identity

F32 = mybir.dt.float32
F32R = mybir.dt.float32r
BF16 = mybir.dt.bfloat16

B, T, De, Do, H, W = 8, 4, 128, 512, 32, 32
HW = H * W            # 1024
THW = T * HW          # 4096
NKT = THW // 128      # 32 k-tiles
NDT = Do // 128       # 4 d-tiles
NH = 2                # n halves of 512
NSPLIT = HW // NH     # 512
SCALE = 1.0 / math.sqrt(De)


def build_nc():
    nc = bacc.Bacc("TRN2", debug=False, num_devices=8, num_swdge_queues=4)

    m_in = nc.dram_tensor("m_in", (T, De, HW), F32, kind="ExternalInput").ap()
    m_out = nc.dram_tensor("m_out", (Do, THW), F32, kind="ExternalInput").ap()
    q_in = nc.dram_tensor("q_in", (De, HW), F32, kind="ExternalInput").ap()
    q_out = nc.dram_tensor("q_out", (Do, HW), F32, kind="ExternalInput").ap()
    p_out = nc.dram_tensor("p", (THW, HW), F32, kind="ExternalOutput").ap()
    mem_out = nc.dram_tensor("mem_out", (2 * Do, HW), F32, kind="ExternalOutput").ap()

    with tile.TileContext(nc) as tc:
        with (
            tc.tile_pool(name="singles", bufs=1) as singles,
            tc.tile_pool(name="mo_stage", bufs=2) as mo_stage,
            tc.tile_pool(name="e_pool", bufs=8) as e_pool,
            tc.tile_pool(name="p_stage", bufs=3) as p_stage,
            tc.tile_pool(name="mem_stage", bufs=2) as mem_stage,
            tc.tile_pool(name="small", bufs=2) as small,
            tc.tile_pool(name="ps_s", bufs=2, space="PSUM") as ps_s,
            tc.tile_pool(name="ps_mem", bufs=2, space="PSUM") as ps_mem,
            tc.tile_pool(name="ps_sums", bufs=1, space="PSUM") as ps_sums,
            tc.tile_pool(name="ps_misc", bufs=1, space="PSUM") as ps_misc,
        ):
            # ---- static tiles -------------------------------------------
            qi = singles.tile([De, HW], F32R)        # queries, 0.5 MB
            nc.sync.dma_start(out=qi, in_=q_in.bitcast(F32R))

            mi_T = singles.tile([De, T, HW], F32R)   # (d, k) keys, 2 MB
            mi_src = m_in.rearrange("t d n -> d t n").bitcast(F32R)
            for t in range(T):
                nc.sync.dma_start(out=mi_T[:, t, :], in_=mi_src[:, t, :])
            mi_T = mi_T.rearrange("d t n -> d (t n)")

            ident_bf = singles.tile([128, 128], BF16)
            make_identity(nc, ident_bf)
            ones_f32 = singles.tile([128, 128], F32)
            nc.vector.memset(ones_f32, 1.0)
            ones_col = singles.tile([128, 1], F32R)  # K-dim ones for sums
            nc.scalar.copy(out=ones_col, in_=ones_f32[:, :1])
            ones_row = singles.tile([1, 128], F32R)  # M-dim ones for bcast
            nc.scalar.copy(out=ones_row, in_=ones_f32[:1, :])
            ones_bf = singles.tile([128, 1], BF16)
            nc.scalar.copy(out=ones_bf, in_=ones_f32[:, :1])

            moT = singles.tile([128, NKT, Do], BF16)  # (k, kt, d), 4 MB

            def trace_mo_transpose(i):
                """Load natural-layout mo d-tile i and PE-transpose its 32
                (128,128) k-blocks into moT."""
                mo_nat = mo_stage.tile([128, THW], BF16, tag="mo_bf")
                nc.gpsimd.dma_start(
                    out=mo_nat, in_=m_out[i * 128 : (i + 1) * 128, :]
                )
                for g in range(NKT // 4):
                    tp = ps_misc.tile([128, 512], BF16, tag="misc")
                    for j in range(4):
                        kt = g * 4 + j
                        nc.tensor.transpose(
                            out=tp[:, j * 128 : (j + 1) * 128],
                            in_=mo_nat[:, kt * 128 : (kt + 1) * 128],
                            identity=ident_bf,
                        )
                    nc.vector.tensor_copy(
                        out=moT[:, g * 4 : (g + 1) * 4, i * 128 : (i + 1) * 128],
                        in_=tp.rearrange("p (j d) -> p j d", j=4),
                    )

            # ---- main: two n-halves, software-pipelined ----------------
            # h0: A-loop (scores/exp/sums, mo transposes as PE filler)
            #     -> recip/bcast -> p-scale h0 (DVE, overlaps h1) -> mem pass a
            # h1: A-loop with h0's mem pass b interleaved as PE filler
            #     -> finish h0 pass b -> recip/bcast -> mem passes -> p-scale
            state = {}

            def a_loop(nh, filler):
                n0 = nh * NSPLIT
                e_groups = [
                    e_pool.tile([128, 4, NSPLIT], F32R, name=f"e_{nh}_{g}", tag="e")
                    for g in range(NKT // 4)
                ]
                ebf_groups = [
                    e_pool.tile([128, 4, NSPLIT], BF16, name=f"ebf_{nh}_{g}", tag="ebf")
                    for g in range(NKT // 4)
                ]
                sums_ps = ps_sums.tile([1, NSPLIT], F32, name=f"sums_{nh}", tag="sums")
                for j in range(NKT // 2):
                    s_ps = ps_s.tile([128, 2, NSPLIT], F32)
                    for i in range(2):
                        kt = j * 2 + i
                        nc.tensor.matmul(
                            out=s_ps[:, i, :],
                            lhsT=mi_T[:, kt * 128 : (kt + 1) * 128],
                            rhs=qi[:, n0 : n0 + NSPLIT],
                            start=True,
                            stop=True,
                        )
                    g, r = (j * 2) // 4, (j * 2) % 4
                    nc.scalar.activation(
                        out=e_groups[g][:, r : r + 2, :],
                        in_=s_ps,
                        func=mybir.ActivationFunctionType.Exp,
                        scale=SCALE,
                    )
                    if j % 2 == 1:
                        g = j // 2
                        nc.vector.tensor_copy(
                            out=ebf_groups[g],
                            in_=e_groups[g].bitcast(F32),
                        )
                        for r in range(4):
                            kt = g * 4 + r
                            nc.tensor.matmul(
                                out=sums_ps,
                                lhsT=ones_bf,
                                rhs=ebf_groups[g][:, r, :],
                                start=(kt == 0),
                                stop=(kt == NKT - 1),
                            )
                    filler(j)
                state[nh] = (e_groups, ebf_groups, sums_ps)

            def b2(nh):
                # reciprocal of the column sums, broadcast to 128 partitions
                sums_ps = state[nh][2]
                r_f32 = small.tile([1, NSPLIT], F32, name=f"r_f32_{nh}", tag="r_f32")
                nc.vector.reciprocal(out=r_f32, in_=sums_ps)
                r_sb = small.tile([1, NSPLIT], F32R, name=f"r_sb_{nh}", tag="r_sb")
                nc.scalar.copy(out=r_sb, in_=r_f32)
                rb_ps = ps_misc.tile([128, NSPLIT], F32, name=f"rb_ps{nh}", tag="misc")
                nc.tensor.matmul(out=rb_ps, lhsT=ones_row, rhs=r_sb, start=True, stop=True)
                rb = small.tile([128, NSPLIT], F32, name=f"rb_{nh}", tag="rb")
                nc.scalar.copy(out=rb, in_=rb_ps)
                return rb

            def mem_pair_alloc(nh, half_d):
                return [
                    ps_mem.tile([128, NSPLIT], F32, name=f"mem_ps{nh}_{half_d}_{i}", tag="mem_ps")
                    for i in range(2)
                ]

            def mem_mm(nh, pair, half_d, kt):
                ebf_groups = state[nh][1]
                for i in range(2):
                    dt = 2 * half_d + i
                    nc.tensor.matmul(
                        out=pair[i],
                        lhsT=moT[:, kt, dt * 128 : (dt + 1) * 128],
                        rhs=ebf_groups[kt // 4][:, kt % 4, :],
                        start=(kt == 0),
                        stop=(kt == NKT - 1),
                    )

            def mem_scale_store(nh, half_d, pair, rb):
                n0 = nh * NSPLIT
                for i in range(2):
                    dt = 2 * half_d + i
                    mst = mem_stage.tile([128, NSPLIT], F32)
                    nc.vector.tensor_mul(out=mst, in0=pair[i], in1=rb)
                    nc.sync.dma_start(
                        out=mem_out[dt * 128 : (dt + 1) * 128, n0 : n0 + NSPLIT],
                        in_=mst,
                    )

            def p_scale(nh, rb):
                n0 = nh * NSPLIT
                e_groups = state[nh][0]
                rb_b = bass.AP(
                    tensor=rb.tensor,
                    offset=rb.offset,
                    ap=[rb.ap[0], [0, 4], rb.ap[1]],
                )
                p_view = p_out.rearrange("(kt part) n -> part kt n", part=128)
                for c in range(NKT // 4):
                    pst = p_stage.tile([128, 4, NSPLIT], F32)
                    nc.vector.tensor_mul(
                        out=pst, in0=e_groups[c].bitcast(F32), in1=rb_b
                    )
                    eng = nc.gpsimd if c % 2 == 0 else nc.sync
                    eng.dma_start(
                        out=p_view[:, c * 4 : (c + 1) * 4, n0 : n0 + NSPLIT],
                        in_=pst,
                    )

            # ---------------- the pipeline ----------------
            a_loop(0, lambda j: trace_mo_transpose(j // 2) if j % 2 == 1 and j <= 7 else None)
            rb0 = b2(0)
            p_scale(0, rb0)          # DVE work, overlaps h1 below
            pair_h0a = mem_pair_alloc(0, 0)
            for kt in range(NKT):
                mem_mm(0, pair_h0a, 0, kt)
            mem_scale_store(0, 0, pair_h0a, rb0)

            # h1 A-loop with h0's second mem pass as PE filler (1-pair lag)
            pair_h0b = mem_pair_alloc(0, 1)

            def h1_filler(j):
                if j >= 1:
                    for kt in (2 * (j - 1), 2 * (j - 1) + 1):
                        mem_mm(0, pair_h0b, 1, kt)

            a_loop(1, h1_filler)
            for kt in (NKT - 2, NKT - 1):
                mem_mm(0, pair_h0b, 1, kt)
            mem_scale_store(0, 1, pair_h0b, rb0)

            rb1 = b2(1)
            p_scale(1, rb1)
            pair_h1a = mem_pair_alloc(1, 0)
            for kt in range(NKT):
                mem_mm(1, pair_h1a, 0, kt)
            mem_scale_store(1, 0, pair_h1a, rb1)
            pair_h1b = mem_pair_alloc(1, 1)
            for kt in range(NKT):
                mem_mm(1, pair_h1b, 1, kt)
            mem_scale_store(1, 1, pair_h1b, rb1)

            # q_out passthrough (DRAM -> DRAM), traced last so it doesn't
            # compete with the input loads for early HBM bandwidth
            nc.sync.dma_start(out=mem_out[Do:, :], in_=q_out)

    nc.compile()
    return nc


_NC_CACHE = None


def _get_nc():
    global _NC_CACHE
    if _NC_CACHE is None:
        _NC_CACHE = build_nc()
    return _NC_CACHE


def kernel(m_in, m_out, q_in, q_out):
    m_in = np.ascontiguousarray(np.asarray(m_in, dtype=np.float32))
    m_out = np.ascontiguousarray(np.asarray(m_out, dtype=np.float32))
    q_in = np.ascontiguousarray(np.asarray(q_in, dtype=np.float32))
    q_out = np.ascontiguousarray(np.asarray(q_out, dtype=np.float32))

    nc = _get_nc()
    in_maps = [
        {
            "m_in": m_in[b].reshape(T, De, HW),
            "m_out": m_out[b].reshape(Do, THW),
            "q_in": q_in[b].reshape(De, HW),
            "q_out": q_out[b].reshape(Do, HW),
        }
        for b in range(B)
    ]

    trace = bool(int(os.environ.get("KERNEL_TRACE", "0")))
    res = bass_utils.run_bass_kernel_spmd(
        nc, in_maps, core_ids=list(range(B)), trace=trace
    )
    kernel.last_results = res

    mem_full = np.stack([res.results[b]["mem_out"] for b in range(B)]).reshape(
        B, 2 * Do, H, W
    )
    p_full = np.stack([res.results[b]["p"] for b in range(B)])
    return (mem_full, p_full)


# revision 31
# speedup vs baseline: 1.0697x; 1.0697x over previous
"""Trainium2 Bass kernel for nn_MemoryCore (sparse_attention).

Reference computation per batch b (B=8, T=4, De=128, Do=512, H=W=32):
    mi   = m_in transposed to (THW=4096, De=128)        # keys
    qi   = q_in as (De=128, HW=1024)                    # queries
    s    = mi @ qi / sqrt(De)                           # (4096, 1024)
    p    = softmax(s, axis=0)                           # over THW
    mo   = m_out raw-reshaped to (Do=512, THW=4096)
    mem  = mo @ p                                       # (512, 1024)
    out  = concat([mem.reshape(512,32,32), q_out])      # (1024, 32, 32)
    returns (out, p)

Sharding: pure data-parallel, one batch per NeuronCore (8 cores).

Kernel strategy (per core):
  - bmm1 computed in (k, n) layout: s[k,n] tiles of (128, 512), lhsT =
    mi_T[:, kblock] (d on partitions), rhs = qi[:, nhalf].  fp32r matmuls
    (1 cycle/row at N=512, ~1e-3 precision - far better than bf16).
  - softmax over k without max subtraction (scores ~ N(0,1); exp is safe
    in fp32).  e = exp(s/sqrt(De)) via ScalarE straight out of PSUM.
  - denominators: ones(128,1)^T @ e accumulated over all 32 k-tiles on PE.
  - bmm2: mem[d,n] accumulated in PSUM over k-tiles; lhsT = moT blocks
    produced by PE transposes of the natural-layout mo.
  - column scaling by 1/sums applied at the end to both p (the big output)
    and mem, using a PE-broadcast (1,512)->(128,512) reciprocal row.
"""

import math
import os
import sys

for _p in ("/opt/trn_rl_repo",):
    if os.path.isdir(_p) and _p not in sys.path:
        sys.path.insert(0, _p)

import numpy as np

import concourse.bass as bass
import concourse.mybir as mybir
import concourse.tile as tile
from concourse import bacc, bass_utils
from concourse.masks import make_identity

F32 = mybir.dt.float32
F32R = mybir.dt.float32r
BF16 = mybir.dt.bfloat16

B, T, De, Do, H, W = 8, 4, 128, 512, 32, 32
HW = H * W            # 1024
THW = T * HW          # 4096
NKT = THW // 128      # 32 k-tiles
NDT = Do // 128       # 4 d-tiles
NH = 2                # n halves of 512
NSPLIT = HW // NH     # 512
SCALE = 1.0 / math.sqrt(De)


def build_nc():
    nc = bacc.Bacc("TRN2", debug=False, num_devices=8)

    m_in = nc.dram_tensor("m_in", (T, De, HW), F32, kind="ExternalInput").ap()
    m_out = nc.dram_tensor("m_out", (Do, THW), F32, kind="ExternalInput").ap()
    q_in = nc.dram_tensor("q_in", (De, HW), F32, kind="ExternalInput").ap()
    q_out = nc.dram_tensor("q_out", (Do, HW), F32, kind="ExternalInput").ap()
    p_out = nc.dram_tensor("p", (THW, HW), F32, kind="ExternalOutput").ap()
    mem_out = nc.dram_tensor("mem_out", (2 * Do, HW), F32, kind="ExternalOutput").ap()

    with tile.TileContext(nc) as tc:
        with (
            tc.tile_pool(name="singles", bufs=1) as singles,
            tc.tile_pool(name="mo_stage", bufs=2) as mo_stage,
            tc.tile_pool(name="e_pool", bufs=8) as e_pool,
            tc.tile_pool(name="p_stage", bufs=3) as p_stage,
            tc.tile_pool(name="mem_stage", bufs=2) as mem_stage,
            tc.tile_pool(name="small", bufs=2) as small,
            tc.tile_pool(name="ps_s", bufs=2, space="PSUM") as ps_s,
            tc.tile_pool(name="ps_mem", bufs=2, space="PSUM") as ps_mem,
            tc.tile_pool(name="ps_sums", bufs=1, space="PSUM") as ps_sums,
            tc.tile_pool(name="ps_misc", bufs=1, space="PSUM") as ps_misc,
        ):
            # ---- static tiles -------------------------------------------
            qi = singles.tile([De, HW], F32R)        # queries, 0.5 MB
            nc.sync.dma_start(out=qi, in_=q_in.bitcast(F32R))

            mi_T = singles.tile([De, T, HW], F32R)   # (d, k) keys, 2 MB
            mi_src = m_in.rearrange("t d n -> d t n").bitcast(F32R)
            for t in range(T):
                nc.sync.dma_start(out=mi_T[:, t, :], in_=mi_src[:, t, :])
            mi_T = mi_T.rearrange("d t n -> d (t n)")

            ident_bf = singles.tile([128, 128], BF16)
            make_identity(nc, ident_bf)
            ones_f32 = singles.tile([128, 128], F32)
            nc.vector.memset(ones_f32, 1.0)
            ones_col = singles.tile([128, 1], F32R)  # K-dim ones for sums
            nc.scalar.copy(out=ones_col, in_=ones_f32[:, :1])
            ones_row = singles.tile([1, 128], F32R)  # M-dim ones for bcast
            nc.scalar.copy(out=ones_row, in_=ones_f32[:1, :])

            moT = singles.tile([128, NKT, Do], BF16)  # (k, kt, d), 4 MB

            def trace_mo_transpose(i):
                """Load natural-layout mo d-tile i and PE-transpose its 32
                (128,128) k-blocks into moT."""
                mo_nat = mo_stage.tile([128, THW], BF16, tag="mo_bf")
                nc.gpsimd.dma_start(
                    out=mo_nat, in_=m_out[i * 128 : (i + 1) * 128, :]
                )
                for g in range(NKT // 4):
                    tp = ps_misc.tile([128, 512], BF16, tag="misc")
                    for j in range(4):
                        kt = g * 4 + j
                        nc.tensor.transpose(
                            out=tp[:, j * 128 : (j + 1) * 128],
                            in_=mo_nat[:, kt * 128 : (kt + 1) * 128],
                            identity=ident_bf,
                        )
                    nc.vector.tensor_copy(
                        out=moT[:, g * 4 : (g + 1) * 4, i * 128 : (i + 1) * 128],
                        in_=tp.rearrange("p (j d) -> p j d", j=4),
                    )

            # ---- main: two n-halves, software-pipelined ----------------
            # h0: A-loop (scores/exp/sums, mo transposes as PE filler)
            #     -> recip/bcast -> p-scale h0 (DVE, overlaps h1) -> mem pass a
            # h1: A-loop with h0's mem pass b interleaved as PE filler
            #     -> finish h0 pass b -> recip/bcast -> mem passes -> p-scale
            state = {}

            def a_loop(nh, filler):
                n0 = nh * NSPLIT
                e_groups = [
                    e_pool.tile([128, 4, NSPLIT], F32R, name=f"e_{nh}_{g}", tag="e")
                    for g in range(NKT // 4)
                ]
                ebf_groups = [
                    e_pool.tile([128, 4, NSPLIT], BF16, name=f"ebf_{nh}_{g}", tag="ebf")
                    for g in range(NKT // 4)
                ]
                sums_ps = ps_sums.tile([1, NSPLIT], F32, name=f"sums_{nh}", tag="sums")
                for j in range(NKT // 2):
                    s_ps = ps_s.tile([128, 2, NSPLIT], F32)
                    for i in range(2):
                        kt = j * 2 + i
                        nc.tensor.matmul(
                            out=s_ps[:, i, :],
                            lhsT=mi_T[:, kt * 128 : (kt + 1) * 128],
                            rhs=qi[:, n0 : n0 + NSPLIT],
                            start=True,
                            stop=True,
                        )
                    g, r = (j * 2) // 4, (j * 2) % 4
                    nc.scalar.activation(
                        out=e_groups[g][:, r : r + 2, :],
                        in_=s_ps,
                        func=mybir.ActivationFunctionType.Exp,
                        scale=SCALE,
                    )
                    if j % 2 == 1:
                        nc.vector.tensor_copy(
                            out=ebf_groups[j // 2],
                            in_=e_groups[j // 2].bitcast(F32),
                        )
                    for i in range(2):
                        kt = j * 2 + i
                        nc.tensor.matmul(
                            out=sums_ps,
                            lhsT=ones_col,
                            rhs=e_groups[kt // 4][:, kt % 4, :],
                            start=(kt == 0),
                            stop=(kt == NKT - 1),
                        )
                    filler(j)
                state[nh] = (e_groups, ebf_groups, sums_ps)

            def b2(nh):
                # reciprocal of the column sums, broadcast to 128 partitions
                sums_ps = state[nh][2]
                r_f32 = small.tile([1, NSPLIT], F32, name=f"r_f32_{nh}", tag="r_f32")
                nc.vector.reciprocal(out=r_f32, in_=sums_ps)
                r_sb = small.tile([1, NSPLIT], F32R, name=f"r_sb_{nh}", tag="r_sb")
                nc.scalar.copy(out=r_sb, in_=r_f32)
                rb_ps = ps_misc.tile([128, NSPLIT], F32, name=f"rb_ps{nh}", tag="misc")
                nc.tensor.matmul(out=rb_ps, lhsT=ones_row, rhs=r_sb, start=True, stop=True)
                rb = small.tile([128, NSPLIT], F32, name=f"rb_{nh}", tag="rb")
                nc.scalar.copy(out=rb, in_=rb_ps)
                return rb

            def mem_pair_alloc(nh, half_d):
                return [
                    ps_mem.tile([128, NSPLIT], F32, name=f"mem_ps{nh}_{half_d}_{i}", tag="mem_ps")
                    for i in range(2)
                ]

            def mem_mm(nh, pair, half_d, kt):
                ebf_groups = state[nh][1]
                for i in range(2):
                    dt = 2 * half_d + i
                    nc.tensor.matmul(
                        out=pair[i],
                        lhsT=moT[:, kt, dt * 128 : (dt + 1) * 128],
                        rhs=ebf_groups[kt // 4][:, kt % 4, :],
                        start=(kt == 0),
                        stop=(kt == NKT - 1),
                    )

            def mem_scale_store(nh, half_d, pair, rb):
                n0 = nh * NSPLIT
                for i in range(2):
                    dt = 2 * half_d + i
                    mst = mem_stage.tile([128, NSPLIT], F32)
                    nc.vector.tensor_mul(out=mst, in0=pair[i], in1=rb)
                    nc.sync.dma_start(
                        out=mem_out[dt * 128 : (dt + 1) * 128, n0 : n0 + NSPLIT],
                        in_=mst,
                    )

            def p_scale(nh, rb):
                n0 = nh * NSPLIT
                e_groups = state[nh][0]
                rb_b = bass.AP(
                    tensor=rb.tensor,
                    offset=rb.offset,
                    ap=[rb.ap[0], [0, 4], rb.ap[1]],
                )
                p_view = p_out.rearrange("(kt part) n -> part kt n", part=128)
                for c in range(NKT // 4):
                    pst = p_stage.tile([128, 4, NSPLIT], F32)
                    nc.vector.tensor_mul(
                        out=pst, in0=e_groups[c].bitcast(F32), in1=rb_b
                    )
                    eng = nc.gpsimd if c % 2 == 0 else nc.sync
                    eng.dma_start(
                        out=p_view[:, c * 4 : (c + 1) * 4, n0 : n0 + NSPLIT],
                        in_=pst,
                    )

            # ---------------- the pipeline ----------------
            a_loop(0, lambda j: trace_mo_transpose(j // 4) if j % 4 == 3 else None)
            rb0 = b2(0)
            p_scale(0, rb0)          # DVE work, overlaps h1 below
            pair_h0a = mem_pair_alloc(0, 0)
            for kt in range(NKT):
                mem_mm(0, pair_h0a, 0, kt)
            mem_scale_store(0, 0, pair_h0a, rb0)

            # h1 A-loop with h0's second mem pass as PE filler (1-pair lag)
            pair_h0b = mem_pair_alloc(0, 1)

            def h1_filler(j):
                if j >= 1:
                    for kt in (2 * (j - 1), 2 * (j - 1) + 1):
                        mem_mm(0, pair_h0b, 1, kt)

            a_loop(1, h1_filler)
            for kt in (NKT - 2, NKT - 1):
                mem_mm(0, pair_h0b, 1, kt)
            mem_scale_store(0, 1, pair_h0b, rb0)

            rb1 = b2(1)
            p_scale(1, rb1)
            pair_h1a = mem_pair_alloc(1, 0)
            for kt in range(NKT):
                mem_mm(1, pair_h1a, 0, kt)
            mem_scale_store(1, 0, pair_h1a, rb1)
            pair_h1b = mem_pair_alloc(1, 1)
            for kt in range(NKT):
                mem_mm(1, pair_h1b, 1, kt)
            mem_scale_store(1, 1, pair_h1b, rb1)

            # q_out passthrough (DRAM -> DRAM), traced last so it doesn't
            # compete with the input loads for early HBM bandwidth
            nc.sync.dma_start(out=mem_out[Do:, :], in_=q_out)

    nc.compile()
    return nc


_NC_CACHE = None


def _get_nc():
    global _NC_CACHE
    if _NC_CACHE is None:
        _NC_CACHE = build_nc()
    return _NC_CACHE


def kernel(m_in, m_out, q_in, q_out):
    m_in = np.ascontiguousarray(np.asarray(m_in, dtype=np.float32))
    m_out = np.ascontiguousarray(np.asarray(m_out, dtype=np.float32))
    q_in = np.ascontiguousarray(np.asarray(q_in, dtype=np.float32))
    q_out = np.ascontiguousarray(np.asarray(q_out, dtype=np.float32))

    nc = _get_nc()
    in_maps = [
        {
            "m_in": m_in[b].reshape(T, De, HW),
            "m_out": m_out[b].reshape(Do, THW),
            "q_in": q_in[b].reshape(De, HW),
            "q_out": q_out[b].reshape(Do, HW),
        }
        for b in range(B)
    ]

    trace = bool(int(os.environ.get("KERNEL_TRACE", "0")))
    res = bass_utils.run_bass_kernel_spmd(
        nc, in_maps, core_ids=list(range(B)), trace=trace
    )
    kernel.last_results = res

    mem_full = np.stack([res.results[b]["mem_out"] for b in range(B)]).reshape(
        B, 2 * Do, H, W
    )
    p_full = np.stack([res.results[b]["p"] for b in range(B)])
    return (mem_full, p_full)


# revision 32
# speedup vs baseline: 1.0801x; 1.0098x over previous
"""Trainium2 Bass kernel for nn_MemoryCore (sparse_attention).

Reference computation per batch b (B=8, T=4, De=128, Do=512, H=W=32):
    mi   = m_in transposed to (THW=4096, De=128)        # keys
    qi   = q_in as (De=128, HW=1024)                    # queries
    s    = mi @ qi / sqrt(De)                           # (4096, 1024)
    p    = softmax(s, axis=0)                           # over THW
    mo   = m_out raw-reshaped to (Do=512, THW=4096)
    mem  = mo @ p                                       # (512, 1024)
    out  = concat([mem.reshape(512,32,32), q_out])      # (1024, 32, 32)
    returns (out, p)

Sharding: pure data-parallel, one batch per NeuronCore (8 cores).

Kernel strategy (per core):
  - bmm1 computed in (k, n) layout: s[k,n] tiles of (128, 512), lhsT =
    mi_T[:, kblock] (d on partitions), rhs = qi[:, nhalf].  fp32r matmuls
    (1 cycle/row at N=512, ~1e-3 precision - far better than bf16).
  - softmax over k without max subtraction (scores ~ N(0,1); exp is safe
    in fp32).  e = exp(s/sqrt(De)) via ScalarE straight out of PSUM.
  - denominators: ones(128,1)^T @ e accumulated over all 32 k-tiles on PE.
  - bmm2: mem[d,n] accumulated in PSUM over k-tiles; lhsT = moT blocks
    produced by PE transposes of the natural-layout mo.
  - column scaling by 1/sums applied at the end to both p (the big output)
    and mem, using a PE-broadcast (1,512)->(128,512) reciprocal row.
"""

import math
import os
import sys

for _p in ("/opt/trn_rl_repo",):
    if os.path.isdir(_p) and _p not in sys.path:
        sys.path.insert(0, _p)

import numpy as np

import concourse.bass as bass
import concourse.mybir as mybir
import concourse.tile as tile
from concourse import bacc, bass_utils
from concourse.masks import make_identity

F32 = mybir.dt.float32
F32R = mybir.dt.float32r
BF16 = mybir.dt.bfloat16

B, T, De, Do, H, W = 8, 4, 128, 512, 32, 32
HW = H * W            # 1024
THW = T * HW          # 4096
NKT = THW // 128      # 32 k-tiles
NDT = Do // 128       # 4 d-tiles
NH = 2                # n halves of 512
NSPLIT = HW // NH     # 512
SCALE = 1.0 / math.sqrt(De)


def build_nc():
    nc = bacc.Bacc("TRN2", debug=False, num_devices=8)

    m_in = nc.dram_tensor("m_in", (T, De, HW), F32, kind="ExternalInput").ap()
    m_out = nc.dram_tensor("m_out", (Do, THW), F32, kind="ExternalInput").ap()
    q_in = nc.dram_tensor("q_in", (De, HW), F32, kind="ExternalInput").ap()
    q_out = nc.dram_tensor("q_out", (Do, HW), F32, kind="ExternalInput").ap()
    p_out = nc.dram_tensor("p", (THW, HW), F32, kind="ExternalOutput").ap()
    mem_out = nc.dram_tensor("mem_out", (2 * Do, HW), F32, kind="ExternalOutput").ap()

    with tile.TileContext(nc) as tc:
        with (
            tc.tile_pool(name="singles", bufs=1) as singles,
            tc.tile_pool(name="mo_stage", bufs=2) as mo_stage,
            tc.tile_pool(name="e_pool", bufs=8) as e_pool,
            tc.tile_pool(name="p_stage", bufs=3) as p_stage,
            tc.tile_pool(name="mem_stage", bufs=2) as mem_stage,
            tc.tile_pool(name="small", bufs=2) as small,
            tc.tile_pool(name="ps_s", bufs=2, space="PSUM") as ps_s,
            tc.tile_pool(name="ps_mem", bufs=2, space="PSUM") as ps_mem,
            tc.tile_pool(name="ps_sums", bufs=1, space="PSUM") as ps_sums,
            tc.tile_pool(name="ps_misc", bufs=1, space="PSUM") as ps_misc,
        ):
            # ---- static tiles -------------------------------------------
            qi = singles.tile([De, HW], F32R)        # queries, 0.5 MB
            nc.sync.dma_start(out=qi, in_=q_in.bitcast(F32R))

            mi_T = singles.tile([De, T, HW], F32R)   # (d, k) keys, 2 MB
            mi_src = m_in.rearrange("t d n -> d t n").bitcast(F32R)
            for t in range(T):
                nc.sync.dma_start(out=mi_T[:, t, :], in_=mi_src[:, t, :])
            mi_T = mi_T.rearrange("d t n -> d (t n)")

            ident_bf = singles.tile([128, 128], BF16)
            make_identity(nc, ident_bf)
            ones_f32 = singles.tile([128, 128], F32)
            nc.vector.memset(ones_f32, 1.0)
            ones_col = singles.tile([128, 1], F32R)  # K-dim ones for sums
            nc.scalar.copy(out=ones_col, in_=ones_f32[:, :1])
            ones_row = singles.tile([1, 128], F32R)  # M-dim ones for bcast
            nc.scalar.copy(out=ones_row, in_=ones_f32[:1, :])

            moT = singles.tile([128, NKT, Do], BF16)  # (k, kt, d), 4 MB

            # HAM warm-up: the PE is DMA-gated for the first ~15us; run
            # dummy matmuls on a memset tile so the array is at full clock
            # when the first real scores arrive
            warm = singles.tile([128, 512], BF16)
            nc.vector.memset(warm, 1.0)
            for w in range(20):
                wp = ps_misc.tile([128, 512], F32, name=f"warm_{w}", tag="misc")
                nc.tensor.matmul(out=wp, lhsT=warm[:, :128], rhs=warm, start=True, stop=True)

            def trace_mo_transpose(i):
                """Load natural-layout mo d-tile i and PE-transpose its 32
                (128,128) k-blocks into moT."""
                mo_nat = mo_stage.tile([128, THW], BF16, tag="mo_bf")
                nc.gpsimd.dma_start(
                    out=mo_nat, in_=m_out[i * 128 : (i + 1) * 128, :]
                )
                for g in range(NKT // 4):
                    tp = ps_misc.tile([128, 512], BF16, tag="misc")
                    for j in range(4):
                        kt = g * 4 + j
                        nc.tensor.transpose(
                            out=tp[:, j * 128 : (j + 1) * 128],
                            in_=mo_nat[:, kt * 128 : (kt + 1) * 128],
                            identity=ident_bf,
                        )
                    nc.vector.tensor_copy(
                        out=moT[:, g * 4 : (g + 1) * 4, i * 128 : (i + 1) * 128],
                        in_=tp.rearrange("p (j d) -> p j d", j=4),
                    )

            # ---- main: two n-halves, software-pipelined ----------------
            # h0: A-loop (scores/exp/sums, mo transposes as PE filler)
            #     -> recip/bcast -> p-scale h0 (DVE, overlaps h1) -> mem pass a
            # h1: A-loop with h0's mem pass b interleaved as PE filler
            #     -> finish h0 pass b -> recip/bcast -> mem passes -> p-scale
            state = {}

            def a_loop(nh, filler):
                n0 = nh * NSPLIT
                e_groups = [
                    e_pool.tile([128, 4, NSPLIT], F32R, name=f"e_{nh}_{g}", tag="e")
                    for g in range(NKT // 4)
                ]
                ebf_groups = [
                    e_pool.tile([128, 4, NSPLIT], BF16, name=f"ebf_{nh}_{g}", tag="ebf")
                    for g in range(NKT // 4)
                ]
                sums_ps = ps_sums.tile([1, NSPLIT], F32, name=f"sums_{nh}", tag="sums")
                for j in range(NKT // 2):
                    s_ps = ps_s.tile([128, 2, NSPLIT], F32)
                    for i in range(2):
                        kt = j * 2 + i
                        nc.tensor.matmul(
                            out=s_ps[:, i, :],
                            lhsT=mi_T[:, kt * 128 : (kt + 1) * 128],
                            rhs=qi[:, n0 : n0 + NSPLIT],
                            start=True,
                            stop=True,
                        )
                    g, r = (j * 2) // 4, (j * 2) % 4
                    nc.scalar.activation(
                        out=e_groups[g][:, r : r + 2, :],
                        in_=s_ps,
                        func=mybir.ActivationFunctionType.Exp,
                        scale=SCALE,
                    )
                    if j % 2 == 1:
                        nc.vector.tensor_copy(
                            out=ebf_groups[j // 2],
                            in_=e_groups[j // 2].bitcast(F32),
                        )
                    for i in range(2):
                        kt = j * 2 + i
                        nc.tensor.matmul(
                            out=sums_ps,
                            lhsT=ones_col,
                            rhs=e_groups[kt // 4][:, kt % 4, :],
                            start=(kt == 0),
                            stop=(kt == NKT - 1),
                        )
                    filler(j)
                state[nh] = (e_groups, ebf_groups, sums_ps)

            def b2(nh):
                # reciprocal of the column sums, broadcast to 128 partitions
                sums_ps = state[nh][2]
                r_f32 = small.tile([1, NSPLIT], F32, name=f"r_f32_{nh}", tag="r_f32")
                nc.vector.reciprocal(out=r_f32, in_=sums_ps)
                r_sb = small.tile([1, NSPLIT], F32R, name=f"r_sb_{nh}", tag="r_sb")
                nc.scalar.copy(out=r_sb, in_=r_f32)
                rb_ps = ps_misc.tile([128, NSPLIT], F32, name=f"rb_ps{nh}", tag="misc")
                nc.tensor.matmul(out=rb_ps, lhsT=ones_row, rhs=r_sb, start=True, stop=True)
                rb = small.tile([128, NSPLIT], F32, name=f"rb_{nh}", tag="rb")
                nc.scalar.copy(out=rb, in_=rb_ps)
                return rb

            def mem_pair_alloc(nh, half_d):
                return [
                    ps_mem.tile([128, NSPLIT], F32, name=f"mem_ps{nh}_{half_d}_{i}", tag="mem_ps")
                    for i in range(2)
                ]

            def mem_mm(nh, pair, half_d, kt):
                ebf_groups = state[nh][1]
                for i in range(2):
                    dt = 2 * half_d + i
                    nc.tensor.matmul(
                        out=pair[i],
                        lhsT=moT[:, kt, dt * 128 : (dt + 1) * 128],
                        rhs=ebf_groups[kt // 4][:, kt % 4, :],
                        start=(kt == 0),
                        stop=(kt == NKT - 1),
                    )

            def mem_scale_store(nh, half_d, pair, rb):
                n0 = nh * NSPLIT
                for i in range(2):
                    dt = 2 * half_d + i
                    mst = mem_stage.tile([128, NSPLIT], F32)
                    nc.vector.tensor_mul(out=mst, in0=pair[i], in1=rb)
                    nc.sync.dma_start(
                        out=mem_out[dt * 128 : (dt + 1) * 128, n0 : n0 + NSPLIT],
                        in_=mst,
                    )

            def p_scale(nh, rb):
                n0 = nh * NSPLIT
                e_groups = state[nh][0]
                rb_b = bass.AP(
                    tensor=rb.tensor,
                    offset=rb.offset,
                    ap=[rb.ap[0], [0, 4], rb.ap[1]],
                )
                p_view = p_out.rearrange("(kt part) n -> part kt n", part=128)
                for c in range(NKT // 4):
                    pst = p_stage.tile([128, 4, NSPLIT], F32)
                    nc.vector.tensor_mul(
                        out=pst, in0=e_groups[c].bitcast(F32), in1=rb_b
                    )
                    eng = nc.gpsimd if c % 2 == 0 else nc.sync
                    eng.dma_start(
                        out=p_view[:, c * 4 : (c + 1) * 4, n0 : n0 + NSPLIT],
                        in_=pst,
                    )

            # ---------------- the pipeline ----------------
            a_loop(0, lambda j: trace_mo_transpose(j // 4) if j % 4 == 3 else None)
            rb0 = b2(0)
            p_scale(0, rb0)          # DVE work, overlaps h1 below
            pair_h0a = mem_pair_alloc(0, 0)
            for kt in range(NKT):
                mem_mm(0, pair_h0a, 0, kt)
            mem_scale_store(0, 0, pair_h0a, rb0)

            # h1 A-loop with h0's second mem pass as PE filler (1-pair lag)
            pair_h0b = mem_pair_alloc(0, 1)

            def h1_filler(j):
                if j >= 1:
                    for kt in (2 * (j - 1), 2 * (j - 1) + 1):
                        mem_mm(0, pair_h0b, 1, kt)

            a_loop(1, h1_filler)
            for kt in (NKT - 2, NKT - 1):
                mem_mm(0, pair_h0b, 1, kt)
            mem_scale_store(0, 1, pair_h0b, rb0)

            rb1 = b2(1)
            p_scale(1, rb1)
            pair_h1a = mem_pair_alloc(1, 0)
            for kt in range(NKT):
                mem_mm(1, pair_h1a, 0, kt)
            mem_scale_store(1, 0, pair_h1a, rb1)
            pair_h1b = mem_pair_alloc(1, 1)
            for kt in range(NKT):
                mem_mm(1, pair_h1b, 1, kt)
            mem_scale_store(1, 1, pair_h1b, rb1)

            # q_out passthrough (DRAM -> DRAM), traced last so it doesn't
            # compete with the input loads for early HBM bandwidth
            nc.sync.dma_start(out=mem_out[Do:, :], in_=q_out)

    nc.compile()
    return nc


_NC_CACHE = None


def _get_nc():
    global _NC_CACHE
    if _NC_CACHE is None:
        _NC_CACHE = build_nc()
    return _NC_CACHE


def kernel(m_in, m_out, q_in, q_out):
    m_in = np.ascontiguousarray(np.asarray(m_in, dtype=np.float32))
    m_out = np.ascontiguousarray(np.asarray(m_out, dtype=np.float32))
    q_in = np.ascontiguousarray(np.asarray(q_in, dtype=np.float32))
    q_out = np.ascontiguousarray(np.asarray(q_out, dtype=np.float32))

    nc = _get_nc()
    in_maps = [
        {
            "m_in": m_in[b].reshape(T, De, HW),
            "m_out": m_out[b].reshape(Do, THW),
            "q_in": q_in[b].reshape(De, HW),
            "q_out": q_out[b].reshape(Do, HW),
        }
        for b in range(B)
    ]

    trace = bool(int(os.environ.get("KERNEL_TRACE", "0")))
    res = bass_utils.run_bass_kernel_spmd(
        nc, in_maps, core_ids=list(range(B)), trace=trace
    )
    kernel.last_results = res

    mem_full = np.stack([res.results[b]["mem_out"] for b in range(B)]).reshape(
        B, 2 * Do, H, W
    )
    p_full = np.stack([res.results[b]["p"] for b in range(B)])
    return (mem_full, p_full)


# revision 33
# speedup vs baseline: 1.1204x; 1.0373x over previous
"""Trainium2 Bass kernel for nn_MemoryCore (sparse_attention).

Reference computation per batch b (B=8, T=4, De=128, Do=512, H=W=32):
    mi   = m_in transposed to (THW=4096, De=128)        # keys
    qi   = q_in as (De=128, HW=1024)                    # queries
    s    = mi @ qi / sqrt(De)                           # (4096, 1024)
    p    = softmax(s, axis=0)                           # over THW
    mo   = m_out raw-reshaped to (Do=512, THW=4096)
    mem  = mo @ p                                       # (512, 1024)
    out  = concat([mem.reshape(512,32,32), q_out])      # (1024, 32, 32)
    returns (out, p)

Sharding: pure data-parallel, one batch per NeuronCore (8 cores).

Kernel strategy (per core):
  - bmm1 computed in (k, n) layout: s[k,n] tiles of (128, 512), lhsT =
    mi_T[:, kblock] (d on partitions), rhs = qi[:, nhalf].  fp32r matmuls
    (1 cycle/row at N=512, ~1e-3 precision - far better than bf16).
  - softmax over k without max subtraction (scores ~ N(0,1); exp is safe
    in fp32).  e = exp(s/sqrt(De)) via ScalarE straight out of PSUM.
  - denominators: ones(128,1)^T @ e accumulated over all 32 k-tiles on PE.
  - bmm2: mem[d,n] accumulated in PSUM over k-tiles; lhsT = moT blocks
    produced by PE transposes of the natural-layout mo.
  - column scaling by 1/sums applied at the end to both p (the big output)
    and mem, using a PE-broadcast (1,512)->(128,512) reciprocal row.
"""

import math
import os
import sys

for _p in ("/opt/trn_rl_repo",):
    if os.path.isdir(_p) and _p not in sys.path:
        sys.path.insert(0, _p)

import numpy as np

import concourse.bass as bass
import concourse.mybir as mybir
import concourse.tile as tile
from concourse import bacc, bass_utils
from concourse.masks import make_identity

F32 = mybir.dt.float32
F32R = mybir.dt.float32r
BF16 = mybir.dt.bfloat16

B, T, De, Do, H, W = 8, 4, 128, 512, 32, 32
HW = H * W            # 1024
THW = T * HW          # 4096
NKT = THW // 128      # 32 k-tiles
NDT = Do // 128       # 4 d-tiles
NH = 2                # n halves of 512
NSPLIT = HW // NH     # 512
SCALE = 1.0 / math.sqrt(De)


def build_nc():
    nc = bacc.Bacc("TRN2", debug=False, num_devices=8)

    m_in = nc.dram_tensor("m_in", (T, De, HW), F32, kind="ExternalInput").ap()
    m_out = nc.dram_tensor("m_out", (Do, THW), F32, kind="ExternalInput").ap()
    q_in = nc.dram_tensor("q_in", (De, HW), F32, kind="ExternalInput").ap()
    q_out = nc.dram_tensor("q_out", (Do, HW), F32, kind="ExternalInput").ap()
    p_out = nc.dram_tensor("p", (THW, HW), F32, kind="ExternalOutput").ap()
    mem_out = nc.dram_tensor("mem_out", (2 * Do, HW), F32, kind="ExternalOutput").ap()

    with tile.TileContext(nc) as tc:
        with (
            tc.tile_pool(name="singles", bufs=1) as singles,
            tc.tile_pool(name="mo_stage", bufs=2) as mo_stage,
            tc.tile_pool(name="e_pool", bufs=8) as e_pool,
            tc.tile_pool(name="p_stage", bufs=3) as p_stage,
            tc.tile_pool(name="mem_stage", bufs=2) as mem_stage,
            tc.tile_pool(name="small", bufs=2) as small,
            tc.tile_pool(name="ps_s", bufs=2, space="PSUM") as ps_s,
            tc.tile_pool(name="ps_mem", bufs=2, space="PSUM") as ps_mem,
            tc.tile_pool(name="ps_sums", bufs=1, space="PSUM") as ps_sums,
            tc.tile_pool(name="ps_misc", bufs=1, space="PSUM") as ps_misc,
        ):
            # ---- static tiles -------------------------------------------
            qi = singles.tile([De, HW], F32R)        # queries, 0.5 MB
            nc.sync.dma_start(out=qi, in_=q_in.bitcast(F32R))

            mi_T = singles.tile([De, T, HW], F32R)   # (d, k) keys, 2 MB
            mi_src = m_in.rearrange("t d n -> d t n").bitcast(F32R)
            for t in range(T):
                nc.sync.dma_start(out=mi_T[:, t, :], in_=mi_src[:, t, :])
            mi_T = mi_T.rearrange("d t n -> d (t n)")

            ident_bf = singles.tile([128, 128], BF16)
            make_identity(nc, ident_bf)
            ones_f32 = singles.tile([128, 128], F32)
            nc.vector.memset(ones_f32, 1.0)
            ones_col = singles.tile([128, 1], F32R)  # K-dim ones for sums
            nc.scalar.copy(out=ones_col, in_=ones_f32[:, :1])
            ones_row = singles.tile([1, 128], F32R)  # M-dim ones for bcast
            nc.scalar.copy(out=ones_row, in_=ones_f32[:1, :])

            moT = singles.tile([128, NKT, Do], BF16)  # (k, kt, d), 4 MB

            def trace_mo_transpose(i):
                """Load natural-layout mo d-tile i and PE-transpose its 32
                (128,128) k-blocks into moT."""
                mo_nat = mo_stage.tile([128, THW], BF16, tag="mo_bf")
                nc.gpsimd.dma_start(
                    out=mo_nat, in_=m_out[i * 128 : (i + 1) * 128, :]
                )
                for g in range(NKT // 4):
                    tp = ps_misc.tile([128, 512], BF16, tag="misc")
                    for j in range(4):
                        kt = g * 4 + j
                        nc.tensor.transpose(
                            out=tp[:, j * 128 : (j + 1) * 128],
                            in_=mo_nat[:, kt * 128 : (kt + 1) * 128],
                            identity=ident_bf,
                        )
                    nc.vector.tensor_copy(
                        out=moT[:, g * 4 : (g + 1) * 4, i * 128 : (i + 1) * 128],
                        in_=tp.rearrange("p (j d) -> p j d", j=4),
                    )

            # ---- main: two n-halves, software-pipelined ----------------
            # h0: A-loop (scores/exp/sums, mo transposes as PE filler)
            #     -> recip/bcast -> p-scale h0 (DVE, overlaps h1) -> mem pass a
            # h1: A-loop with h0's mem pass b interleaved as PE filler
            #     -> finish h0 pass b -> recip/bcast -> mem passes -> p-scale
            state = {}

            def a_loop(nh, filler):
                n0 = nh * NSPLIT
                e_groups = [
                    e_pool.tile([128, 4, NSPLIT], F32R, name=f"e_{nh}_{g}", tag="e")
                    for g in range(NKT // 4)
                ]
                ebf_groups = [
                    e_pool.tile([128, 4, NSPLIT], BF16, name=f"ebf_{nh}_{g}", tag="ebf")
                    for g in range(NKT // 4)
                ]
                sums_ps = ps_sums.tile([1, NSPLIT], F32, name=f"sums_{nh}", tag="sums")
                for j in range(NKT // 2):
                    s_ps = ps_s.tile([128, 2, NSPLIT], F32)
                    for i in range(2):
                        kt = j * 2 + i
                        nc.tensor.matmul(
                            out=s_ps[:, i, :],
                            lhsT=mi_T[:, kt * 128 : (kt + 1) * 128],
                            rhs=qi[:, n0 : n0 + NSPLIT],
                            start=True,
                            stop=True,
                        )
                    g, r = (j * 2) // 4, (j * 2) % 4
                    nc.scalar.activation(
                        out=e_groups[g][:, r : r + 2, :],
                        in_=s_ps,
                        func=mybir.ActivationFunctionType.Exp,
                        scale=SCALE,
                    )
                    if j % 2 == 1:
                        nc.vector.tensor_copy(
                            out=ebf_groups[j // 2],
                            in_=e_groups[j // 2].bitcast(F32),
                        )
                    for i in range(2):
                        kt = j * 2 + i
                        nc.tensor.matmul(
                            out=sums_ps,
                            lhsT=ones_col,
                            rhs=e_groups[kt // 4][:, kt % 4, :],
                            start=(kt == 0),
                            stop=(kt == NKT - 1),
                        )
                    filler(j)
                state[nh] = (e_groups, ebf_groups, sums_ps)

            def b2(nh):
                # reciprocal of the column sums, broadcast to 128 partitions
                sums_ps = state[nh][2]
                r_f32 = small.tile([1, NSPLIT], F32, name=f"r_f32_{nh}", tag="r_f32")
                nc.vector.reciprocal(out=r_f32, in_=sums_ps)
                r_sb = small.tile([1, NSPLIT], F32R, name=f"r_sb_{nh}", tag="r_sb")
                nc.scalar.copy(out=r_sb, in_=r_f32)
                rb_ps = ps_misc.tile([128, NSPLIT], F32, name=f"rb_ps{nh}", tag="misc")
                nc.tensor.matmul(out=rb_ps, lhsT=ones_row, rhs=r_sb, start=True, stop=True)
                rb = small.tile([128, NSPLIT], F32, name=f"rb_{nh}", tag="rb")
                nc.scalar.copy(out=rb, in_=rb_ps)
                return rb

            def mem_pair_alloc(nh, half_d):
                return [
                    ps_mem.tile([128, NSPLIT], F32, name=f"mem_ps{nh}_{half_d}_{i}", tag="mem_ps")
                    for i in range(2)
                ]

            def mem_mm(nh, pair, half_d, kt):
                ebf_groups = state[nh][1]
                for i in range(2):
                    dt = 2 * half_d + i
                    nc.tensor.matmul(
                        out=pair[i],
                        lhsT=moT[:, kt, dt * 128 : (dt + 1) * 128],
                        rhs=ebf_groups[kt // 4][:, kt % 4, :],
                        start=(kt == 0),
                        stop=(kt == NKT - 1),
                    )

            def mem_scale_store(nh, half_d, pair, rb):
                n0 = nh * NSPLIT
                for i in range(2):
                    dt = 2 * half_d + i
                    mst = mem_stage.tile([128, NSPLIT], F32)
                    nc.vector.tensor_mul(out=mst, in0=pair[i], in1=rb)
                    nc.sync.dma_start(
                        out=mem_out[dt * 128 : (dt + 1) * 128, n0 : n0 + NSPLIT],
                        in_=mst,
                    )

            def p_scale(nh, rb):
                n0 = nh * NSPLIT
                e_groups = state[nh][0]
                rb_b = bass.AP(
                    tensor=rb.tensor,
                    offset=rb.offset,
                    ap=[rb.ap[0], [0, 4], rb.ap[1]],
                )
                p_view = p_out.rearrange("(kt part) n -> part kt n", part=128)
                for c in range(NKT // 4):
                    pst = p_stage.tile([128, 4, NSPLIT], F32)
                    nc.vector.tensor_mul(
                        out=pst, in0=e_groups[c].bitcast(F32), in1=rb_b
                    )
                    eng = nc.gpsimd if c % 2 == 0 else nc.sync
                    eng.dma_start(
                        out=p_view[:, c * 4 : (c + 1) * 4, n0 : n0 + NSPLIT],
                        in_=pst,
                    )

            # ---------------- the pipeline ----------------
            a_loop(0, lambda j: trace_mo_transpose(j // 4) if j % 4 == 3 else None)
            rb0 = b2(0)
            p_scale(0, rb0)          # DVE work, overlaps h1 below
            pair_h0a = mem_pair_alloc(0, 0)
            for kt in range(NKT):
                mem_mm(0, pair_h0a, 0, kt)
            mem_scale_store(0, 0, pair_h0a, rb0)

            # h1 A-loop with h0's second mem pass as PE filler (1-pair lag)
            pair_h0b = mem_pair_alloc(0, 1)

            def h1_filler(j):
                if j >= 1:
                    for kt in (2 * (j - 1), 2 * (j - 1) + 1):
                        mem_mm(0, pair_h0b, 1, kt)

            a_loop(1, h1_filler)
            for kt in (NKT - 2, NKT - 1):
                mem_mm(0, pair_h0b, 1, kt)
            mem_scale_store(0, 1, pair_h0b, rb0)

            rb1 = b2(1)
            p_scale(1, rb1)
            pair_h1a = mem_pair_alloc(1, 0)
            for kt in range(NKT):
                mem_mm(1, pair_h1a, 0, kt)
            mem_scale_store(1, 0, pair_h1a, rb1)
            pair_h1b = mem_pair_alloc(1, 1)
            for kt in range(NKT):
                mem_mm(1, pair_h1b, 1, kt)
            mem_scale_store(1, 1, pair_h1b, rb1)

            # q_out passthrough (DRAM -> DRAM), traced last so it doesn't
            # compete with the input loads for early HBM bandwidth
            nc.sync.dma_start(out=mem_out[Do:, :], in_=q_out)

    nc.compile()
    return nc


_NC_CACHE = None


def _get_nc():
    global _NC_CACHE
    if _NC_CACHE is None:
        _NC_CACHE = build_nc()
    return _NC_CACHE


def kernel(m_in, m_out, q_in, q_out):
    m_in = np.ascontiguousarray(np.asarray(m_in, dtype=np.float32))
    m_out = np.ascontiguousarray(np.asarray(m_out, dtype=np.float32))
    q_in = np.ascontiguousarray(np.asarray(q_in, dtype=np.float32))
    q_out = np.ascontiguousarray(np.asarray(q_out, dtype=np.float32))

    nc = _get_nc()
    in_maps = [
        {
            "m_in": m_in[b].reshape(T, De, HW),
            "m_out": m_out[b].reshape(Do, THW),
            "q_in": q_in[b].reshape(De, HW),
            "q_out": q_out[b].reshape(Do, HW),
        }
        for b in range(B)
    ]

    trace = bool(int(os.environ.get("KERNEL_TRACE", "0")))
    res = bass_utils.run_bass_kernel_spmd(
        nc, in_maps, core_ids=list(range(B)), trace=trace
    )
    kernel.last_results = res

    mem_full = np.stack([res.results[b]["mem_out"] for b in range(B)]).reshape(
        B, 2 * Do, H, W
    )
    p_full = np.stack([res.results[b]["p"] for b in range(B)])
    return (mem_full, p_full)
